# revision 1
# baseline (speedup 1.0000x reference)
"""GCN single-head message passing on 8 Trainium2 NeuronCores.

Strategy (dst-sharded, fully local per core — no collectives):
  - Each core owns 1/8 of the destination nodes (contiguous range) and all
    edges pointing into them (~200k edges/core).
  - Linearity: out = relu(norm_dst * segsum_dst((feature @ W.T * norm_src)[src]))
             = relu(norm_dst * (segsum_dst((feature * norm_src)[src]) @ W.T))
    so we aggregate (feature * norm_src) rows (128-dim) per dst first, then
    apply W.T once per dst block of 128.
  - The per-edge gather of 256B bf16 feature rows uses gpsimd.dma_gather
    (SWDGE descriptor gather).  int16 gather indices only reach 32768 rows,
    so the node table is split in 4 chunks of 25000 rows and edges are
    bucketed by (chunk, dst-block).
  - Scatter/segment-sum is done on the PE: for each 128-edge tile a
    selection matrix S[e, d] = (dst_rel[e] == d) is built on DVE via
    is_equal against a constant iota tile, then matmul(lhsT=gathered_feats,
    rhs=S) accumulates transposed per-block sums agg.T[f, d] in PSUM.
  - Per-(chunk, block) edge-slot sizes are equalized across the 8 cores
    (max, rounded to 16) so all cores share one SPMD program; padding edges
    gather row 0 with dst_rel = -1 (zero column in S).
  - Stage 2 per block: matmul(lhsT=agg.T, rhs=W.T) -> out[d, o] in PSUM,
    then one DVE tensor_scalar (x*norm_dst, max 0) writes the staged output.
Host work is structural only: dtype staging (bf16 cast of feature*norm),
edge bucketing / index + dst_rel array construction, output unblocking.
"""

import sys

import numpy as np

if "/opt/trn_rl_repo" not in sys.path:
    sys.path.insert(0, "/opt/trn_rl_repo")

# ---------------------------------------------------------------- constants
N_NODES = 100000
N_EDGES = 1600000
F = 128              # in feats
O = 64               # out feats
NCORES = 8
ND = N_NODES // NCORES       # dst nodes per core (12500)
NCHUNK = 4
CHUNK = N_NODES // NCHUNK    # src rows per gather chunk (25000, < 32768)
BLK = 128                    # dst nodes per block (PSUM partitions)
NBLK = -(-ND // BLK)         # blocks per core (98)
WINB = 12                   # dst blocks per window (gather granularity)
SLOT_ALIGN = 8               # per-(chunk, block) slot rounding
MIN_SLOT0 = 16               # chunk-0 slots forced nonzero => block coverage


def _round_up(x, m):
    return (x + m - 1) // m * m


# ---------------------------------------------------------------- host prep
def _prep(feature, norm, weight, edge_src, edge_dst):
    """Build per-core staged inputs + the static schedule (shared by cores)."""
    from concourse import mybir

    bf16 = mybir.dt.np(mybir.dt.bfloat16)

    table = (feature.astype(np.float32) * norm.astype(np.float32)).astype(bf16)

    edge_src = np.asarray(edge_src).astype(np.int64)
    edge_dst = np.asarray(edge_dst).astype(np.int64)

    # ---- bucket edges per (core, chunk, block)
    per_core = []   # (src_sorted_by_bucket, dstrel_sorted, counts[4*NBLK])
    nbuck = NCHUNK * NBLK
    for k in range(NCORES):
        m = (edge_dst // ND) == k
        es = edge_src[m]
        ed = edge_dst[m] - k * ND
        c = es // CHUNK
        b = ed // BLK
        key = c * NBLK + b
        order = np.argsort(key, kind="stable")
        es, ed, key = es[order], ed[order], key[order]
        counts = np.bincount(key, minlength=nbuck)
        per_core.append((es, ed, counts))

    counts_all = np.stack([pc[2] for pc in per_core])          # [8, 4*98]
    slots = counts_all.max(axis=0)
    slots = _round_up(slots, SLOT_ALIGN)
    slots = slots.reshape(NCHUNK, NBLK)
    slots[0] = np.maximum(slots[0], MIN_SLOT0)                 # coverage
    # window layout
    windows = []
    b0 = 0
    while b0 < NBLK:
        windows.append((b0, min(b0 + WINB, NBLK)))
        b0 += WINB

    # ---- static schedule shared by all cores
    # per (w, c): T_cw (padded edges), idx col offset, rel col offset, gcols
    # per (w, b): list of (c, j, lo, hi) matmul sub-ranges
    sched = []
    icol = 0   # int16 columns consumed in idx_all
    rcol = 0   # bf16 columns consumed in rel_all
    for (bw0, bw1) in windows:
        gather = []
        mm = {b: [] for b in range(bw0, bw1)}
        for c in range(NCHUNK):
            # round to full 128-columns: every row a matmul reads is written
            T = _round_up(int(slots[c, bw0:bw1].sum()), 128)
            gcols = T // 128
            pos = 0
            for b in range(bw0, bw1):
                s = int(slots[c, b])
                p0, p1 = pos, pos + s
                j0, j1 = p0 // 128, (p1 - 1) // 128
                for j in range(j0, j1 + 1):
                    mm[b].append((c, j))
                pos = p1
            gather.append(dict(c=c, T=T, icol=icol, rcol=rcol, gcols=gcols))
            icol += T // 16
            rcol += gcols
        sched.append(dict(b0=bw0, b1=bw1, gather=gather, mm=mm))
    icols_total, rcols_total = icol, rcol
    gmax = [max(g["gcols"] for w in sched for g in w["gather"]
                if g["c"] == c) for c in range(NCHUNK)]

    # ---- per-core staged arrays
    in_maps = []
    wt = np.ascontiguousarray(weight.astype(np.float32).T)     # [128, 64]
    iota = np.broadcast_to(np.arange(WINB * BLK, dtype=np.float32),
                           (128, WINB * BLK)).astype(np.float16)
    for k in range(NCORES):
        es, ed, counts = per_core[k]
        starts = np.zeros(nbuck + 1, dtype=np.int64)
        np.cumsum(counts, out=starts[1:])
        idx_all = np.zeros((128, icols_total), dtype=np.int16)
        rel_all = np.full((128, rcols_total), -1.0, dtype=np.float32)
        for w in sched:
            for g in w["gather"]:
                c, T, ic, rc = g["c"], g["T"], g["icol"], g["rcol"]
                loc_i = np.zeros(T, dtype=np.int16)
                loc_r = np.full(T, -1.0, dtype=np.float32)
                pos = 0
                for b in range(w["b0"], w["b1"]):
                    bk = c * NBLK + b
                    n = int(counts[bk])
                    sl = int(slots[c, b])
                    sseg = slice(starts[bk], starts[bk] + n)
                    loc_i[pos:pos + n] = (es[sseg] - c * CHUNK).astype(np.int16)
                    # dst index relative to the WINDOW start (fp16-exact)
                    loc_r[pos:pos + n] = (ed[sseg] - w["b0"] * BLK).astype(
                        np.float32)
                    pos += sl
                idx_all[:, ic:ic + T // 16] = np.tile(
                    loc_i.reshape(T // 16, 16).T, (8, 1))
                gc = g["gcols"]
                relpad = np.full(gc * 128, -1.0, dtype=np.float32)
                relpad[:T] = loc_r
                rel_all[:, rc:rc + gc] = relpad.reshape(gc, 128).T
        nrm = np.zeros((128, NBLK), dtype=np.float32)
        nloc = norm.reshape(-1)[k * ND:(k + 1) * ND].astype(np.float32)
        npad = np.zeros(NBLK * BLK, dtype=np.float32)
        npad[:ND] = nloc
        nrm[:, :] = npad.reshape(NBLK, BLK).T
        in_maps.append(dict(table=table, idx=idx_all,
                            rel=rel_all.astype(np.float16), iota=iota,
                            wt=wt, nrm=nrm))
    meta = dict(sched=sched, icols=icols_total, rcols=rcols_total, gmax=gmax)
    return meta, in_maps


# ------------------------------------------------------------ device program
def _build(meta, reps=1, skip_mm=False, skip_s2=False):
    from contextlib import ExitStack

    import concourse.bacc as bacc
    import concourse.tile as tile
    from concourse import mybir

    f32 = mybir.dt.float32
    bf16 = mybir.dt.bfloat16
    fp16 = mybir.dt.float16
    i16 = mybir.dt.int16

    nc = bacc.Bacc("TRN2", target_bir_lowering=False, debug=False)
    t_table = nc.declare_dram_parameter("table", [N_NODES, F], bf16, False)
    t_idx = nc.declare_dram_parameter("idx", [128, meta["icols"]], i16, False)
    t_rel = nc.declare_dram_parameter("rel", [128, meta["rcols"]], fp16, False)
    t_iota = nc.declare_dram_parameter("iota", [128, WINB * BLK], fp16, False)
    t_wt = nc.declare_dram_parameter("wt", [F, O], f32, False)
    t_nrm = nc.declare_dram_parameter("nrm", [128, NBLK], f32, False)
    t_out = nc.declare_dram_parameter("out", [128, NBLK * O], f32, True)

    mult = mybir.AluOpType.mult
    amax = mybir.AluOpType.max
    iseq = mybir.AluOpType.is_equal

    with ExitStack() as ctx:
        tc = ctx.enter_context(tile.TileContext(nc))
        const = ctx.enter_context(tc.tile_pool(name="const", bufs=1))
        gpool = ctx.enter_context(tc.tile_pool(name="gp", bufs=2))
        spool = ctx.enter_context(tc.tile_pool(name="sp", bufs=24))
        aggp = ctx.enter_context(tc.tile_pool(name="agg", bufs=3))
        apsum = ctx.enter_context(tc.tile_pool(name="aps", bufs=5, space="PSUM"))
        opsum = ctx.enter_context(tc.tile_pool(name="ops", bufs=2, space="PSUM"))

        idx_sb = const.tile([128, meta["icols"]], i16)
        rel_sb = const.tile([128, meta["rcols"]], fp16)
        iota_sb = const.tile([128, WINB * BLK], fp16)
        wt_sb = const.tile([F, O], f32)
        nrm_sb = const.tile([128, NBLK], f32)
        stage = const.tile([128, NBLK * O], f32)
        nc.sync.dma_start(out=idx_sb[:], in_=t_idx[:])
        nc.sync.dma_start(out=rel_sb[:], in_=t_rel[:])
        nc.sync.dma_start(out=iota_sb[:], in_=t_iota[:])
        nc.sync.dma_start(out=wt_sb[:], in_=t_wt[:])
        nc.sync.dma_start(out=nrm_sb[:], in_=t_nrm[:])
        if skip_mm or skip_s2:
            nc.vector.memset(stage[:], 0.0)   # keep out DMA readable

        for _rep in range(reps):
          for w in meta["sched"]:
            gtiles = {}
            for g in w["gather"]:
                c, T, gc = g["c"], g["T"], g["gcols"]
                gt = gpool.tile([128, meta["gmax"][c], F], bf16, tag=f"g{c}")
                nc.gpsimd.dma_gather(
                    gt[:, :gc, :],
                    t_table[c * CHUNK:(c + 1) * CHUNK, :],
                    idx_sb[:, g["icol"]:g["icol"] + T // 16],
                    num_idxs=T,
                    num_idxs_reg=T,
                    elem_size=F,
                    # >1024 idxs overflow the single-packet SWDGE path on HW
                    single_packet=False,
                )
                gtiles[c] = (gt, g["rcol"])
            if skip_mm:
                continue
            for b in range(w["b0"], w["b1"]):
                ranges = w["mm"][b]
                r = b - w["b0"]
                ps = apsum.tile([128, BLK], f32)
                for i, (c, j) in enumerate(ranges):
                    st = spool.tile([128, BLK], bf16, tag="s")
                    rc = gtiles[c][1]
                    nc.vector.tensor_tensor(
                        out=st[:],
                        in0=rel_sb[:, rc + j:rc + j + 1].to_broadcast(
                            [128, BLK]),
                        in1=iota_sb[:, r * BLK:(r + 1) * BLK],
                        op=iseq,
                    )
                    nc.tensor.matmul(
                        ps[:],
                        lhsT=gtiles[c][0][:, j, :],
                        rhs=st[:],
                        start=(i == 0),
                        stop=(i == len(ranges) - 1),
                    )
                aggT = aggp.tile([128, BLK], f32)
                nc.any.tensor_copy(out=aggT[:], in_=ps[:])
                if skip_s2:
                    continue
                op = opsum.tile([128, O], f32)
                nc.tensor.matmul(op[:], lhsT=aggT[:], rhs=wt_sb[:],
                                 start=True, stop=True)
                nc.vector.tensor_scalar(
                    out=stage[:, b * O:(b + 1) * O],
                    in0=op[:],
                    scalar1=nrm_sb[:, b:b + 1],
                    scalar2=0.0,
                    op0=mult,
                    op1=amax,
                )
        nc.sync.dma_start(out=t_out[:], in_=stage[:])
    nc.compile()
    return nc


# ----------------------------------------------------------------- entry
TRACE = False      # set True (e.g. from test.py) to profile; result in LAST
LAST = None


def kernel(feature, norm, weight, edge_src, edge_dst):
    from concourse.bass_utils import run_bass_kernel_spmd

    meta, in_maps = _prep(feature, norm, weight, edge_src, edge_dst)
    nc = _build(meta)
    res = run_bass_kernel_spmd(nc, in_maps, list(range(NCORES)), trace=TRACE)
    globals()["LAST"] = res
    outs = []
    for k in range(NCORES):
        blocked = np.asarray(res.results[k]["out"], dtype=np.float32)
        ob = blocked.reshape(128, NBLK, O).transpose(1, 0, 2)
        outs.append(ob.reshape(NBLK * BLK, O)[:ND])
    return np.concatenate(outs, axis=0)



# revision 12
# speedup vs baseline: 3.4765x; 3.4765x over previous
"""GCN single-head message passing on 8 Trainium2 NeuronCores.

Strategy (dst-sharded, fully local per core — no collectives):
  - Each core owns 1/8 of the destination nodes (contiguous range) and all
    edges pointing into them (~200k edges/core).
  - Linearity: out = relu(norm_dst * segsum_dst((feature @ W.T * norm_src)[src]))
             = relu(norm_dst * (segsum_dst((feature * norm_src)[src]) @ W.T))
    so we aggregate (feature * norm_src) rows (128-dim) per dst first, then
    apply W.T once per dst block of 128.
  - The per-edge gather of 256B bf16 feature rows uses gpsimd.dma_gather
    (SWDGE descriptor gather).  int16 gather indices only reach 32768 rows,
    so the node table is split in 4 chunks of 25000 rows and edges are
    bucketed by (chunk, dst-block).
  - Scatter/segment-sum is done on the PE: for each 128-edge tile a
    selection matrix S[e, d] = (dst_rel[e] == d) is built on DVE via
    is_equal against a constant iota tile, then matmul(lhsT=gathered_feats,
    rhs=S) accumulates transposed per-block sums agg.T[f, d] in PSUM.
  - Per-(chunk, block) edge-slot sizes are equalized across the 8 cores
    (max, rounded to 16) so all cores share one SPMD program; padding edges
    gather row 0 with dst_rel = -1 (zero column in S).
  - Stage 2 per block: matmul(lhsT=agg.T, rhs=W.T) -> out[d, o] in PSUM,
    then one DVE tensor_scalar (x*norm_dst, max 0) writes the staged output.
Host work is structural only: dtype staging (bf16 cast of feature*norm),
edge bucketing / index + dst_rel array construction, output unblocking.
"""

import sys

import numpy as np

if "/opt/trn_rl_repo" not in sys.path:
    sys.path.insert(0, "/opt/trn_rl_repo")

# ---------------------------------------------------------------- constants
N_NODES = 100000
N_EDGES = 1600000
F = 128              # in feats
O = 64               # out feats
NCORES = 8
ND = N_NODES // NCORES       # dst nodes per core (12500)
NCHUNK = 4
CHUNK = N_NODES // NCHUNK    # src rows per gather chunk (25000, < 32768)
BLK = 128                    # dst nodes per block (PSUM partitions)
NBLK = -(-ND // BLK)         # blocks per core (98)
WINB = 12                   # dst blocks per window (gather granularity)
SLOT_ALIGN = 8               # per-(chunk, block) slot rounding
MIN_SLOT0 = 16               # chunk-0 slots forced nonzero => block coverage


def _round_up(x, m):
    return (x + m - 1) // m * m


# ---------------------------------------------------------------- host prep
def _prep(feature, norm, weight, edge_src, edge_dst):
    """Build per-core staged inputs + the static schedule (shared by cores)."""
    from concourse import mybir

    bf16 = mybir.dt.np(mybir.dt.bfloat16)

    table = (feature.astype(np.float32) * norm.astype(np.float32)).astype(bf16)

    edge_src = np.asarray(edge_src).astype(np.int64)
    edge_dst = np.asarray(edge_dst).astype(np.int64)

    # ---- bucket edges per (core, chunk, block)
    per_core = []   # (src_sorted_by_bucket, dstrel_sorted, counts[4*NBLK])
    nbuck = NCHUNK * NBLK
    for k in range(NCORES):
        m = (edge_dst // ND) == k
        es = edge_src[m]
        ed = edge_dst[m] - k * ND
        c = es // CHUNK
        b = ed // BLK
        key = c * NBLK + b
        order = np.argsort(key, kind="stable")
        es, ed, key = es[order], ed[order], key[order]
        counts = np.bincount(key, minlength=nbuck)
        per_core.append((es, ed, counts))

    counts_all = np.stack([pc[2] for pc in per_core])          # [8, 4*98]
    slots = counts_all.max(axis=0)
    slots = _round_up(slots, SLOT_ALIGN)
    slots = slots.reshape(NCHUNK, NBLK)
    slots[0] = np.maximum(slots[0], MIN_SLOT0)                 # coverage
    # window layout
    windows = []
    b0 = 0
    while b0 < NBLK:
        windows.append((b0, min(b0 + WINB, NBLK)))
        b0 += WINB

    # ---- static schedule shared by all cores
    # per (w, c): T_cw (padded edges), idx col offset, rel col offset, gcols
    # per (w, b): list of (c, j0, K) matmul tile-runs (K consecutive tiles)
    sched = []
    icol = 0   # int16 columns consumed in idx_all
    rcol = 0   # bf16 columns consumed in rel_all
    for (bw0, bw1) in windows:
        gather = []
        mm = {b: [] for b in range(bw0, bw1)}
        for c in range(NCHUNK):
            # round to full 128-columns: every row a matmul reads is written
            T = _round_up(int(slots[c, bw0:bw1].sum()), 128)
            gcols = T // 128
            pos = 0
            for b in range(bw0, bw1):
                s = int(slots[c, b])
                p0, p1 = pos, pos + s
                j0, j1 = p0 // 128, (p1 - 1) // 128
                mm[b].append((c, j0, j1 - j0 + 1))
                pos = p1
            gather.append(dict(c=c, T=T, icol=icol, rcol=rcol, gcols=gcols))
            icol += T // 16
            rcol += gcols
        sched.append(dict(b0=bw0, b1=bw1, gather=gather, mm=mm))
    icols_total, rcols_total = icol, rcol
    gmax = [max(g["gcols"] for w in sched for g in w["gather"]
                if g["c"] == c) for c in range(NCHUNK)]
    kmax = max(K for w in sched for rs in w["mm"].values() for (_, _, K) in rs)

    # ---- per-core staged arrays
    in_maps = []
    wt = np.ascontiguousarray(weight.astype(np.float32).T)     # [128, 64]
    iota = np.broadcast_to(np.arange(WINB * BLK, dtype=np.float32),
                           (128, WINB * BLK)).astype(np.float16)
    iota = np.ascontiguousarray(iota).reshape(128, WINB, BLK)
    for k in range(NCORES):
        es, ed, counts = per_core[k]
        starts = np.zeros(nbuck + 1, dtype=np.int64)
        np.cumsum(counts, out=starts[1:])
        idx_all = np.zeros((128, icols_total), dtype=np.int16)
        rel_all = np.full((128, rcols_total), -1.0, dtype=np.float32)
        for w in sched:
            for g in w["gather"]:
                c, T, ic, rc = g["c"], g["T"], g["icol"], g["rcol"]
                loc_i = np.zeros(T, dtype=np.int16)
                loc_r = np.full(T, -1.0, dtype=np.float32)
                pos = 0
                for b in range(w["b0"], w["b1"]):
                    bk = c * NBLK + b
                    n = int(counts[bk])
                    sl = int(slots[c, b])
                    sseg = slice(starts[bk], starts[bk] + n)
                    loc_i[pos:pos + n] = (es[sseg] - c * CHUNK).astype(np.int16)
                    # dst index relative to the WINDOW start (fp16-exact)
                    loc_r[pos:pos + n] = (ed[sseg] - w["b0"] * BLK).astype(
                        np.float32)
                    pos += sl
                idx_all[:, ic:ic + T // 16] = np.tile(
                    loc_i.reshape(T // 16, 16).T, (8, 1))
                gc = g["gcols"]
                relpad = np.full(gc * 128, -1.0, dtype=np.float32)
                relpad[:T] = loc_r
                rel_all[:, rc:rc + gc] = relpad.reshape(gc, 128).T
        nrm = np.zeros((128, NBLK), dtype=np.float32)
        nloc = norm.reshape(-1)[k * ND:(k + 1) * ND].astype(np.float32)
        npad = np.zeros(NBLK * BLK, dtype=np.float32)
        npad[:ND] = nloc
        nrm[:, :] = npad.reshape(NBLK, BLK).T
        in_maps.append(dict(table=table, idx=idx_all,
                            rel=rel_all.astype(np.float16), iota=iota,
                            wt=wt, nrm=nrm))
    meta = dict(sched=sched, icols=icols_total, rcols=rcols_total, gmax=gmax,
                kmax=kmax)
    return meta, in_maps


# ------------------------------------------------------------ device program
def _build(meta, reps=1, skip_mm=False, skip_s2=False, tsub=1024, nq=4):
    from contextlib import ExitStack

    import concourse.bacc as bacc
    import concourse.tile as tile
    from concourse import mybir

    f32 = mybir.dt.float32
    bf16 = mybir.dt.bfloat16
    fp16 = mybir.dt.float16
    i16 = mybir.dt.int16

    nc = bacc.Bacc("TRN2", target_bir_lowering=False, debug=False,
                   num_swdge_queues=nq)
    t_table = nc.declare_dram_parameter("table", [N_NODES, F], bf16, False)
    t_idx = nc.declare_dram_parameter("idx", [128, meta["icols"]], i16, False)
    t_rel = nc.declare_dram_parameter("rel", [128, meta["rcols"]], fp16, False)
    t_iota = nc.declare_dram_parameter("iota", [128, WINB, BLK], fp16, False)
    t_wt = nc.declare_dram_parameter("wt", [F, O], f32, False)
    t_nrm = nc.declare_dram_parameter("nrm", [128, NBLK], f32, False)
    t_out = nc.declare_dram_parameter("out", [128, NBLK * O], f32, True)

    mult = mybir.AluOpType.mult
    amax = mybir.AluOpType.max
    iseq = mybir.AluOpType.is_equal

    with ExitStack() as ctx:
        tc = ctx.enter_context(tile.TileContext(nc))
        const = ctx.enter_context(tc.tile_pool(name="const", bufs=1))
        gpool = ctx.enter_context(tc.tile_pool(name="gp", bufs=2))
        spool = ctx.enter_context(tc.tile_pool(name="sp", bufs=6))
        aggp = ctx.enter_context(tc.tile_pool(name="agg", bufs=3))
        apsum = ctx.enter_context(tc.tile_pool(name="aps", bufs=5, space="PSUM"))
        opsum = ctx.enter_context(tc.tile_pool(name="ops", bufs=2, space="PSUM"))

        idx_sb = const.tile([128, meta["icols"]], i16)
        rel_sb = const.tile([128, meta["rcols"]], fp16)
        iota_sb = const.tile([128, WINB, BLK], fp16)
        wt_sb = const.tile([F, O], f32)
        nrm_sb = const.tile([128, NBLK], f32)
        stage = const.tile([128, NBLK * O], f32)
        nc.sync.dma_start(out=idx_sb[:], in_=t_idx[:])
        nc.sync.dma_start(out=rel_sb[:], in_=t_rel[:])
        nc.sync.dma_start(out=iota_sb[:], in_=t_iota[:])
        nc.sync.dma_start(out=wt_sb[:], in_=t_wt[:])
        nc.sync.dma_start(out=nrm_sb[:], in_=t_nrm[:])
        if skip_mm or skip_s2:
            nc.vector.memset(stage[:], 0.0)   # keep out DMA readable

        qctr = 0
        for _rep in range(reps):
          for w in meta["sched"]:
            gtiles = {}
            for g in w["gather"]:
                c, T, gc = g["c"], g["T"], g["gcols"]
                gt = gpool.tile([128, meta["gmax"][c], F], bf16, tag=f"g{c}")
                # sub-calls sized ~ the per-queue descriptor-ring carveout,
                # rotated across SWDGE queues so all rings stay fed and the
                # DMA engines drain them concurrently
                for d0 in range(0, T, tsub):
                    d1 = min(T, d0 + tsub)
                    nc.gpsimd.dma_gather(
                        gt[:, d0 // 128:d1 // 128, :],
                        t_table[c * CHUNK:(c + 1) * CHUNK, :],
                        idx_sb[:, g["icol"] + d0 // 16:g["icol"] + d1 // 16],
                        num_idxs=d1 - d0,
                        num_idxs_reg=d1 - d0,
                        elem_size=F,
                        # >1024 idxs overflow single-packet SWDGE on HW
                        single_packet=False,
                        queue_num=qctr % nq,
                    )
                    qctr += 1
                gtiles[c] = (gt, g["rcol"])
            if skip_mm:
                continue
            for b in range(w["b0"], w["b1"]):
                runs = w["mm"][b]
                r = b - w["b0"]
                nmm = sum(K for (_, _, K) in runs)
                ps = apsum.tile([128, BLK], f32)
                i = 0
                for (c, j0, K) in runs:
                    st = spool.tile([128, meta["kmax"], BLK], bf16, tag="s")
                    rc = gtiles[c][1]
                    nc.vector.tensor_tensor(
                        out=st[:, :K, :],
                        in0=rel_sb[:, rc + j0:rc + j0 + K].to_broadcast(
                            [128, K, BLK]),
                        in1=iota_sb[:, r:r + 1, :].to_broadcast([128, K, BLK]),
                        op=iseq,
                    )
                    for k in range(K):
                        nc.tensor.matmul(
                            ps[:],
                            lhsT=gtiles[c][0][:, j0 + k, :],
                            rhs=st[:, k, :],
                            start=(i == 0),
                            stop=(i == nmm - 1),
                        )
                        i += 1
                aggT = aggp.tile([128, BLK], f32)
                nc.any.tensor_copy(out=aggT[:], in_=ps[:])
                if skip_s2:
                    continue
                op = opsum.tile([128, O], f32)
                nc.tensor.matmul(op[:], lhsT=aggT[:], rhs=wt_sb[:],
                                 start=True, stop=True)
                nc.vector.tensor_scalar(
                    out=stage[:, b * O:(b + 1) * O],
                    in0=op[:],
                    scalar1=nrm_sb[:, b:b + 1],
                    scalar2=0.0,
                    op0=mult,
                    op1=amax,
                )
        nc.sync.dma_start(out=t_out[:], in_=stage[:])
    nc.compile()
    return nc


# ----------------------------------------------------------------- entry
TRACE = False      # set True (e.g. from test.py) to profile; result in LAST
LAST = None


def kernel(feature, norm, weight, edge_src, edge_dst):
    from concourse.bass_utils import run_bass_kernel_spmd

    meta, in_maps = _prep(feature, norm, weight, edge_src, edge_dst)
    nc = _build(meta)
    res = run_bass_kernel_spmd(nc, in_maps, list(range(NCORES)), trace=TRACE)
    globals()["LAST"] = res
    outs = []
    for k in range(NCORES):
        blocked = np.asarray(res.results[k]["out"], dtype=np.float32)
        ob = blocked.reshape(128, NBLK, O).transpose(1, 0, 2)
        outs.append(ob.reshape(NBLK * BLK, O)[:ND])
    return np.concatenate(outs, axis=0)



# revision 20
# speedup vs baseline: 3.7186x; 1.0696x over previous
"""GCN single-head message passing on 8 Trainium2 NeuronCores.

Strategy (dst-sharded, fully local per core — no collectives):
  - Each core owns 1/8 of the destination nodes (contiguous range) and all
    edges pointing into them (~200k edges/core).
  - Linearity: out = relu(norm_dst * segsum_dst((feature @ W.T * norm_src)[src]))
             = relu(norm_dst * (segsum_dst((feature * norm_src)[src]) @ W.T))
    so we aggregate (feature * norm_src) rows (128-dim) per dst first, then
    apply W.T once per dst block of 128.
  - The per-edge gather of 256B bf16 feature rows uses gpsimd.dma_gather
    (SWDGE descriptor gather).  int16 gather indices only reach 32768 rows,
    so the node table is split in 4 chunks of 25000 rows and edges are
    bucketed by (chunk, dst-block).
  - Scatter/segment-sum is done on the PE: for each 128-edge tile a
    selection matrix S[e, d] = (dst_rel[e] == d) is built on DVE via
    is_equal against a constant iota tile, then matmul(lhsT=gathered_feats,
    rhs=S) accumulates transposed per-block sums agg.T[f, d] in PSUM.
  - Per-(chunk, block) edge-slot sizes are equalized across the 8 cores
    (max, rounded to 16) so all cores share one SPMD program; padding edges
    gather row 0 with dst_rel = -1 (zero column in S).
  - Stage 2 per block: matmul(lhsT=agg.T, rhs=W.T) -> out[d, o] in PSUM,
    then one DVE tensor_scalar (x*norm_dst, max 0) writes the staged output.
Host work is structural only: dtype staging (bf16 cast of feature*norm),
edge bucketing / index + dst_rel array construction, output unblocking.
"""

import sys

import numpy as np

if "/opt/trn_rl_repo" not in sys.path:
    sys.path.insert(0, "/opt/trn_rl_repo")

# ---------------------------------------------------------------- constants
N_NODES = 100000
N_EDGES = 1600000
F = 128              # in feats
O = 64               # out feats
NCORES = 8
ND = N_NODES // NCORES       # dst nodes per core (12500)
NCHUNK = 4
CHUNK = N_NODES // NCHUNK    # src rows per gather chunk (25000, < 32768)
BLK = 128                    # dst nodes per block (PSUM partitions)
NBLK = -(-ND // BLK)         # blocks per core (98)
WINB = 7                    # dst blocks per window (gather granularity)
SLOT_ALIGN = 8               # per-(chunk, block) slot rounding
MIN_SLOT0 = 16               # chunk-0 slots forced nonzero => block coverage


def _round_up(x, m):
    return (x + m - 1) // m * m


# ---------------------------------------------------------------- host prep
def _prep(feature, norm, weight, edge_src, edge_dst):
    """Build per-core staged inputs + the static schedule (shared by cores)."""
    from concourse import mybir

    bf16 = mybir.dt.np(mybir.dt.bfloat16)

    table = (feature.astype(np.float32) * norm.astype(np.float32)).astype(bf16)

    edge_src = np.asarray(edge_src).astype(np.int64)
    edge_dst = np.asarray(edge_dst).astype(np.int64)

    # ---- bucket edges per (core, chunk, block)
    per_core = []   # (src_sorted_by_bucket, dstrel_sorted, counts[4*NBLK])
    nbuck = NCHUNK * NBLK
    for k in range(NCORES):
        m = (edge_dst // ND) == k
        es = edge_src[m]
        ed = edge_dst[m] - k * ND
        c = es // CHUNK
        b = ed // BLK
        key = c * NBLK + b
        order = np.argsort(key, kind="stable")
        es, ed, key = es[order], ed[order], key[order]
        counts = np.bincount(key, minlength=nbuck)
        per_core.append((es, ed, counts))

    counts_all = np.stack([pc[2] for pc in per_core])          # [8, 4*98]
    slots = counts_all.max(axis=0)
    slots = _round_up(slots, SLOT_ALIGN)
    slots = slots.reshape(NCHUNK, NBLK)
    slots[0] = np.maximum(slots[0], MIN_SLOT0)                 # coverage
    # window layout
    windows = []
    b0 = 0
    while b0 < NBLK:
        windows.append((b0, min(b0 + WINB, NBLK)))
        b0 += WINB

    # ---- static schedule shared by all cores
    # per (w, c): T_cw (padded edges), idx col offset, rel col offset, gcols
    # per (w, b): list of (c, j0, K) matmul tile-runs (K consecutive tiles)
    sched = []
    icol = 0   # int16 columns consumed in idx_all
    rcol = 0   # bf16 columns consumed in rel_all
    for (bw0, bw1) in windows:
        gather = []
        mm = {b: [] for b in range(bw0, bw1)}
        for c in range(NCHUNK):
            # round to full 128-columns: every row a matmul reads is written
            T = _round_up(int(slots[c, bw0:bw1].sum()), 128)
            gcols = T // 128
            pos = 0
            for b in range(bw0, bw1):
                s = int(slots[c, b])
                p0, p1 = pos, pos + s
                j0, j1 = p0 // 128, (p1 - 1) // 128
                mm[b].append((c, j0, j1 - j0 + 1))
                pos = p1
            gather.append(dict(c=c, T=T, icol=icol, rcol=rcol, gcols=gcols))
            icol += T // 16
            rcol += gcols
        sched.append(dict(b0=bw0, b1=bw1, gather=gather, mm=mm))
    icols_total, rcols_total = icol, rcol
    gmax = [max(g["gcols"] for w in sched for g in w["gather"]
                if g["c"] == c) for c in range(NCHUNK)]
    kmax = max(K for w in sched for rs in w["mm"].values() for (_, _, K) in rs)

    # ---- per-core staged arrays
    in_maps = []
    wt = np.ascontiguousarray(weight.astype(np.float32).T)     # [128, 64]
    iota = np.broadcast_to(np.arange(WINB * BLK, dtype=np.float32),
                           (128, WINB * BLK)).astype(np.float16)
    iota = np.ascontiguousarray(iota).reshape(128, WINB, BLK)
    for k in range(NCORES):
        es, ed, counts = per_core[k]
        starts = np.zeros(nbuck + 1, dtype=np.int64)
        np.cumsum(counts, out=starts[1:])
        idx_all = np.zeros((128, icols_total), dtype=np.int16)
        rel_all = np.full((128, rcols_total), -1.0, dtype=np.float32)
        for w in sched:
            for g in w["gather"]:
                c, T, ic, rc = g["c"], g["T"], g["icol"], g["rcol"]
                loc_i = np.zeros(T, dtype=np.int16)
                loc_r = np.full(T, -1.0, dtype=np.float32)
                pos = 0
                for b in range(w["b0"], w["b1"]):
                    bk = c * NBLK + b
                    n = int(counts[bk])
                    sl = int(slots[c, b])
                    sseg = slice(starts[bk], starts[bk] + n)
                    loc_i[pos:pos + n] = (es[sseg] - c * CHUNK).astype(np.int16)
                    # dst index relative to the WINDOW start (fp16-exact)
                    loc_r[pos:pos + n] = (ed[sseg] - w["b0"] * BLK).astype(
                        np.float32)
                    pos += sl
                idx_all[:, ic:ic + T // 16] = np.tile(
                    loc_i.reshape(T // 16, 16).T, (8, 1))
                gc = g["gcols"]
                relpad = np.full(gc * 128, -1.0, dtype=np.float32)
                relpad[:T] = loc_r
                rel_all[:, rc:rc + gc] = relpad.reshape(gc, 128).T
        nrm = np.zeros((128, NBLK), dtype=np.float32)
        nloc = norm.reshape(-1)[k * ND:(k + 1) * ND].astype(np.float32)
        npad = np.zeros(NBLK * BLK, dtype=np.float32)
        npad[:ND] = nloc
        nrm[:, :] = npad.reshape(NBLK, BLK).T
        in_maps.append(dict(table=table, idx=idx_all,
                            rel=rel_all.astype(np.float16), iota=iota,
                            wt=wt, nrm=nrm))
    meta = dict(sched=sched, icols=icols_total, rcols=rcols_total, gmax=gmax,
                kmax=kmax)
    return meta, in_maps


# ------------------------------------------------------------ device program
def _build(meta, reps=1, skip_mm=False, skip_s2=False, skip_eq=False,
           skip_pe=False, tsub=1024, nq=4):
    from contextlib import ExitStack

    import concourse.bacc as bacc
    import concourse.tile as tile
    from concourse import mybir

    f32 = mybir.dt.float32
    bf16 = mybir.dt.bfloat16
    fp16 = mybir.dt.float16
    i16 = mybir.dt.int16

    nc = bacc.Bacc("TRN2", target_bir_lowering=False, debug=False,
                   num_swdge_queues=nq)
    t_table = nc.declare_dram_parameter("table", [N_NODES, F], bf16, False)
    t_idx = nc.declare_dram_parameter("idx", [128, meta["icols"]], i16, False)
    t_rel = nc.declare_dram_parameter("rel", [128, meta["rcols"]], fp16, False)
    t_iota = nc.declare_dram_parameter("iota", [128, WINB, BLK], fp16, False)
    t_wt = nc.declare_dram_parameter("wt", [F, O], f32, False)
    t_nrm = nc.declare_dram_parameter("nrm", [128, NBLK], f32, False)
    t_out = nc.declare_dram_parameter("out", [128, NBLK * O], f32, True)

    mult = mybir.AluOpType.mult
    amax = mybir.AluOpType.max
    iseq = mybir.AluOpType.is_equal

    with ExitStack() as ctx:
        tc = ctx.enter_context(tile.TileContext(nc))
        const = ctx.enter_context(tc.tile_pool(name="const", bufs=1))
        gpool = ctx.enter_context(tc.tile_pool(name="gp", bufs=2))
        spool = ctx.enter_context(tc.tile_pool(name="sp", bufs=16))
        aggp = ctx.enter_context(tc.tile_pool(name="agg", bufs=6))
        apsum = ctx.enter_context(tc.tile_pool(name="aps", bufs=6, space="PSUM"))
        opsum = ctx.enter_context(tc.tile_pool(name="ops", bufs=2, space="PSUM"))

        idx_sb = const.tile([128, meta["icols"]], i16)
        rel_sb = const.tile([128, meta["rcols"]], fp16)
        iota_sb = const.tile([128, WINB, BLK], fp16)
        wt_sb = const.tile([F, O], f32)
        nrm_sb = const.tile([128, NBLK], f32)
        stage = const.tile([128, NBLK * O], f32)
        nc.sync.dma_start(out=idx_sb[:], in_=t_idx[:])
        nc.sync.dma_start(out=rel_sb[:], in_=t_rel[:])
        nc.sync.dma_start(out=iota_sb[:], in_=t_iota[:])
        nc.sync.dma_start(out=wt_sb[:], in_=t_wt[:])
        nc.sync.dma_start(out=nrm_sb[:], in_=t_nrm[:])
        if skip_mm or skip_s2 or skip_pe:
            nc.vector.memset(stage[:], 0.0)   # keep out DMA readable
        st_static = None
        if skip_eq:
            st_static = const.tile([128, meta["kmax"], BLK], bf16)
            nc.vector.memset(st_static[:], 0.0)

        qctr = 0
        for _rep in range(reps):
          for w in meta["sched"]:
            gtiles = {}
            for g in w["gather"]:
                c, T, gc = g["c"], g["T"], g["gcols"]
                gt = gpool.tile([128, meta["gmax"][c], F], bf16, tag=f"g{c}")
                # sub-calls sized ~ the per-queue descriptor-ring carveout,
                # rotated across SWDGE queues so all rings stay fed and the
                # DMA engines drain them concurrently
                for d0 in range(0, T, tsub):
                    d1 = min(T, d0 + tsub)
                    nc.gpsimd.dma_gather(
                        gt[:, d0 // 128:d1 // 128, :],
                        t_table[c * CHUNK:(c + 1) * CHUNK, :],
                        idx_sb[:, g["icol"] + d0 // 16:g["icol"] + d1 // 16],
                        num_idxs=d1 - d0,
                        num_idxs_reg=d1 - d0,
                        elem_size=F,
                        # >1024 idxs overflow single-packet SWDGE on HW
                        single_packet=False,
                        queue_num=qctr % nq,
                    )
                    qctr += 1
                gtiles[c] = (gt, g["rcol"])
            if skip_mm:
                continue
            for b in range(w["b0"], w["b1"]):
                runs = w["mm"][b]
                r = b - w["b0"]
                nmm = sum(K for (_, _, K) in runs)
                ps = apsum.tile([128, BLK], f32)
                i = 0
                for (c, j0, K) in runs:
                    if skip_eq:
                        st = st_static
                    else:
                        st = spool.tile([128, meta["kmax"], BLK], bf16,
                                        tag="s")
                        rc = gtiles[c][1]
                        nc.vector.tensor_tensor(
                            out=st[:, :K, :],
                            in0=rel_sb[:, rc + j0:rc + j0 + K].to_broadcast(
                                [128, K, BLK]),
                            in1=iota_sb[:, r:r + 1, :].to_broadcast(
                                [128, K, BLK]),
                            op=iseq,
                        )
                    if skip_pe:
                        i += K
                        continue
                    for k in range(K):
                        nc.tensor.matmul(
                            ps[:],
                            lhsT=gtiles[c][0][:, j0 + k, :],
                            rhs=st[:, k, :],
                            start=(i == 0),
                            stop=(i == nmm - 1),
                        )
                        i += 1
                if skip_pe:
                    continue
                aggT = aggp.tile([128, BLK], f32)
                nc.any.tensor_copy(out=aggT[:], in_=ps[:])
                if skip_s2:
                    continue
                op = opsum.tile([128, O], f32)
                nc.tensor.matmul(op[:], lhsT=aggT[:], rhs=wt_sb[:],
                                 start=True, stop=True)
                nc.vector.tensor_scalar(
                    out=stage[:, b * O:(b + 1) * O],
                    in0=op[:],
                    scalar1=nrm_sb[:, b:b + 1],
                    scalar2=0.0,
                    op0=mult,
                    op1=amax,
                )
        nc.sync.dma_start(out=t_out[:], in_=stage[:])
    nc.compile()
    return nc


# ----------------------------------------------------------------- entry
TRACE = False      # set True (e.g. from test.py) to profile; result in LAST
LAST = None


def kernel(feature, norm, weight, edge_src, edge_dst):
    from concourse.bass_utils import run_bass_kernel_spmd

    meta, in_maps = _prep(feature, norm, weight, edge_src, edge_dst)
    nc = _build(meta)
    res = run_bass_kernel_spmd(nc, in_maps, list(range(NCORES)), trace=TRACE)
    globals()["LAST"] = res
    outs = []
    for k in range(NCORES):
        blocked = np.asarray(res.results[k]["out"], dtype=np.float32)
        ob = blocked.reshape(128, NBLK, O).transpose(1, 0, 2)
        outs.append(ob.reshape(NBLK * BLK, O)[:ND])
    return np.concatenate(outs, axis=0)



# revision 23
# speedup vs baseline: 4.0900x; 1.0999x over previous
"""GCN single-head message passing on 8 Trainium2 NeuronCores.

Strategy (dst-sharded, fully local per core — no collectives):
  - Each core owns 1/8 of the destination nodes (contiguous range) and all
    edges pointing into them (~200k edges/core).
  - Linearity: out = relu(norm_dst * segsum_dst((feature @ W.T * norm_src)[src]))
             = relu(norm_dst * (segsum_dst((feature * norm_src)[src]) @ W.T))
    so we aggregate (feature * norm_src) rows (128-dim) per dst first, then
    apply W.T once per dst block of 128.
  - The per-edge gather of 256B bf16 feature rows uses gpsimd.dma_gather
    (SWDGE descriptor gather).  int16 gather indices only reach 32768 rows,
    so the node table is split in 4 chunks of 25000 rows and edges are
    bucketed by (chunk, dst-block).
  - Scatter/segment-sum is done on the PE: for each 128-edge tile a
    selection matrix S[e, d] = (dst_rel[e] == d) is built on DVE via
    is_equal against a constant iota tile, then matmul(lhsT=gathered_feats,
    rhs=S) accumulates transposed per-block sums agg.T[f, d] in PSUM.
  - Per-(chunk, block) edge-slot sizes are equalized across the 8 cores
    (max, rounded to 16) so all cores share one SPMD program; padding edges
    gather row 0 with dst_rel = -1 (zero column in S).
  - Stage 2 per block: matmul(lhsT=agg.T, rhs=W.T) -> out[d, o] in PSUM,
    then one DVE tensor_scalar (x*norm_dst, max 0) writes the staged output.
Host work is structural only: dtype staging (bf16 cast of feature*norm),
edge bucketing / index + dst_rel array construction, output unblocking.
"""

import sys

import numpy as np

if "/opt/trn_rl_repo" not in sys.path:
    sys.path.insert(0, "/opt/trn_rl_repo")

# ---------------------------------------------------------------- constants
N_NODES = 100000
N_EDGES = 1600000
F = 128              # in feats
O = 64               # out feats
NCORES = 8
ND = N_NODES // NCORES       # dst nodes per core (12500)
NCHUNK = 4
CHUNK = N_NODES // NCHUNK    # src rows per gather chunk (25000, < 32768)
BLK = 128                    # dst nodes per block (PSUM partitions)
NBLK = -(-ND // BLK)         # blocks per core (98)
WINB = 7                    # dst blocks per window (gather granularity)
SLOT_ALIGN = 8               # per-(chunk, block) slot rounding
MIN_SLOT0 = 16               # chunk-0 slots forced nonzero => block coverage


def _round_up(x, m):
    return (x + m - 1) // m * m


# ---------------------------------------------------------------- host prep
def _prep(feature, norm, weight, edge_src, edge_dst):
    """Build per-core staged inputs + the static schedule (shared by cores)."""
    from concourse import mybir

    bf16 = mybir.dt.np(mybir.dt.bfloat16)

    table = (feature.astype(np.float32) * norm.astype(np.float32)).astype(bf16)

    edge_src = np.asarray(edge_src).astype(np.int64)
    edge_dst = np.asarray(edge_dst).astype(np.int64)

    # ---- bucket edges per (core, chunk, block)
    per_core = []   # (src_sorted_by_bucket, dstrel_sorted, counts[4*NBLK])
    nbuck = NCHUNK * NBLK
    for k in range(NCORES):
        m = (edge_dst // ND) == k
        es = edge_src[m]
        ed = edge_dst[m] - k * ND
        c = es // CHUNK
        b = ed // BLK
        key = c * NBLK + b
        order = np.argsort(key, kind="stable")
        es, ed, key = es[order], ed[order], key[order]
        counts = np.bincount(key, minlength=nbuck)
        per_core.append((es, ed, counts))

    counts_all = np.stack([pc[2] for pc in per_core])          # [8, 4*98]
    slots = counts_all.max(axis=0)
    slots = _round_up(slots, SLOT_ALIGN)
    slots = slots.reshape(NCHUNK, NBLK)
    slots[0] = np.maximum(slots[0], MIN_SLOT0)                 # coverage
    # window layout
    windows = []
    b0 = 0
    while b0 < NBLK:
        windows.append((b0, min(b0 + WINB, NBLK)))
        b0 += WINB

    # ---- static schedule shared by all cores
    # per (w, c): T_cw (padded edges), idx col offset, rel col offset, gcols
    # per (w, b): list of (c, j0, K) matmul tile-runs (K consecutive tiles)
    sched = []
    icol = 0   # int16 columns consumed in idx_all
    rcol = 0   # bf16 columns consumed in rel_all
    for (bw0, bw1) in windows:
        gather = []
        mm = {b: [] for b in range(bw0, bw1)}
        for c in range(NCHUNK):
            # round to full 128-columns: every row a matmul reads is written
            T = _round_up(int(slots[c, bw0:bw1].sum()), 128)
            gcols = T // 128
            pos = 0
            for b in range(bw0, bw1):
                s = int(slots[c, b])
                p0, p1 = pos, pos + s
                j0, j1 = p0 // 128, (p1 - 1) // 128
                mm[b].append((c, j0, j1 - j0 + 1))
                pos = p1
            gather.append(dict(c=c, T=T, icol=icol, rcol=rcol, gcols=gcols))
            icol += T // 16
            rcol += gcols
        sched.append(dict(b0=bw0, b1=bw1, gather=gather, mm=mm))
    icols_total, rcols_total = icol, rcol
    gmax = [max(g["gcols"] for w in sched for g in w["gather"]
                if g["c"] == c) for c in range(NCHUNK)]
    kmax = max(K for w in sched for rs in w["mm"].values() for (_, _, K) in rs)

    # ---- per-core staged arrays
    in_maps = []
    wt = np.ascontiguousarray(weight.astype(np.float32).T)     # [128, 64]
    iota = np.broadcast_to(np.arange(WINB * BLK, dtype=np.float32),
                           (128, WINB * BLK)).astype(np.float16)
    iota = np.ascontiguousarray(iota).reshape(128, WINB, BLK)
    for k in range(NCORES):
        es, ed, counts = per_core[k]
        starts = np.zeros(nbuck + 1, dtype=np.int64)
        np.cumsum(counts, out=starts[1:])
        idx_all = np.zeros((128, icols_total), dtype=np.int16)
        rel_all = np.full((128, rcols_total), -1.0, dtype=np.float32)
        for w in sched:
            for g in w["gather"]:
                c, T, ic, rc = g["c"], g["T"], g["icol"], g["rcol"]
                loc_i = np.zeros(T, dtype=np.int16)
                loc_r = np.full(T, -1.0, dtype=np.float32)
                pos = 0
                for b in range(w["b0"], w["b1"]):
                    bk = c * NBLK + b
                    n = int(counts[bk])
                    sl = int(slots[c, b])
                    sseg = slice(starts[bk], starts[bk] + n)
                    loc_i[pos:pos + n] = (es[sseg] - c * CHUNK).astype(np.int16)
                    # dst index relative to the WINDOW start (fp16-exact)
                    loc_r[pos:pos + n] = (ed[sseg] - w["b0"] * BLK).astype(
                        np.float32)
                    pos += sl
                idx_all[:, ic:ic + T // 16] = np.tile(
                    loc_i.reshape(T // 16, 16).T, (8, 1))
                gc = g["gcols"]
                relpad = np.full(gc * 128, -1.0, dtype=np.float32)
                relpad[:T] = loc_r
                rel_all[:, rc:rc + gc] = relpad.reshape(gc, 128).T
        nrm = np.zeros((128, NBLK), dtype=np.float32)
        nloc = norm.reshape(-1)[k * ND:(k + 1) * ND].astype(np.float32)
        npad = np.zeros(NBLK * BLK, dtype=np.float32)
        npad[:ND] = nloc
        nrm[:, :] = npad.reshape(NBLK, BLK).T
        in_maps.append(dict(table=table, idx=idx_all,
                            rel=rel_all.astype(np.float16), iota=iota,
                            wt=wt, nrm=nrm))
    meta = dict(sched=sched, icols=icols_total, rcols=rcols_total, gmax=gmax,
                kmax=kmax)
    return meta, in_maps


# ------------------------------------------------------------ device program
def _build(meta, reps=1, skip_mm=False, skip_s2=False, skip_eq=False,
           skip_pe=False, tsub=1024, nq=4, s_fp8=True):
    from contextlib import ExitStack

    import concourse.bacc as bacc
    import concourse.tile as tile
    from concourse import mybir

    f32 = mybir.dt.float32
    bf16 = mybir.dt.bfloat16
    fp16 = mybir.dt.float16
    i16 = mybir.dt.int16
    sdt = mybir.dt.float8e4 if s_fp8 else bf16   # S one-hot entries (exact)

    nc = bacc.Bacc("TRN2", target_bir_lowering=False, debug=False,
                   num_swdge_queues=nq)
    t_table = nc.declare_dram_parameter("table", [N_NODES, F], bf16, False)
    t_idx = nc.declare_dram_parameter("idx", [128, meta["icols"]], i16, False)
    t_rel = nc.declare_dram_parameter("rel", [128, meta["rcols"]], fp16, False)
    t_iota = nc.declare_dram_parameter("iota", [128, WINB, BLK], fp16, False)
    t_wt = nc.declare_dram_parameter("wt", [F, O], f32, False)
    t_nrm = nc.declare_dram_parameter("nrm", [128, NBLK], f32, False)
    t_out = nc.declare_dram_parameter("out", [128, NBLK * O], f32, True)

    mult = mybir.AluOpType.mult
    amax = mybir.AluOpType.max
    iseq = mybir.AluOpType.is_equal

    with ExitStack() as ctx:
        tc = ctx.enter_context(tile.TileContext(nc))
        const = ctx.enter_context(tc.tile_pool(name="const", bufs=1))
        gpool = ctx.enter_context(tc.tile_pool(name="gp", bufs=2))
        spool = ctx.enter_context(tc.tile_pool(name="sp", bufs=16))
        aggp = ctx.enter_context(tc.tile_pool(name="agg", bufs=6))
        apsum = ctx.enter_context(tc.tile_pool(name="aps", bufs=6, space="PSUM"))
        opsum = ctx.enter_context(tc.tile_pool(name="ops", bufs=2, space="PSUM"))

        idx_sb = const.tile([128, meta["icols"]], i16)
        rel_sb = const.tile([128, meta["rcols"]], fp16)
        iota_sb = const.tile([128, WINB, BLK], fp16)
        wt_sb = const.tile([F, O], f32)
        nrm_sb = const.tile([128, NBLK], f32)
        stage = const.tile([128, NBLK * O], f32)
        nc.sync.dma_start(out=idx_sb[:], in_=t_idx[:])
        nc.sync.dma_start(out=rel_sb[:], in_=t_rel[:])
        nc.sync.dma_start(out=iota_sb[:], in_=t_iota[:])
        nc.sync.dma_start(out=wt_sb[:], in_=t_wt[:])
        nc.sync.dma_start(out=nrm_sb[:], in_=t_nrm[:])
        if skip_mm or skip_s2 or skip_pe:
            nc.vector.memset(stage[:], 0.0)   # keep out DMA readable
        st_static = None
        if skip_eq:
            st_static = const.tile([128, meta["kmax"], BLK], sdt)
            nc.vector.memset(st_static[:], 0.0)

        qctr = 0
        for _rep in range(reps):
          for w in meta["sched"]:
            gtiles = {}
            for g in w["gather"]:
                c, T, gc = g["c"], g["T"], g["gcols"]
                gt = gpool.tile([128, meta["gmax"][c], F], bf16, tag=f"g{c}")
                # sub-calls sized ~ the per-queue descriptor-ring carveout,
                # rotated across SWDGE queues so all rings stay fed and the
                # DMA engines drain them concurrently
                for d0 in range(0, T, tsub):
                    d1 = min(T, d0 + tsub)
                    nc.gpsimd.dma_gather(
                        gt[:, d0 // 128:d1 // 128, :],
                        t_table[c * CHUNK:(c + 1) * CHUNK, :],
                        idx_sb[:, g["icol"] + d0 // 16:g["icol"] + d1 // 16],
                        num_idxs=d1 - d0,
                        num_idxs_reg=d1 - d0,
                        elem_size=F,
                        # >1024 idxs overflow single-packet SWDGE on HW
                        single_packet=False,
                        queue_num=qctr % nq,
                    )
                    qctr += 1
                gtiles[c] = (gt, g["rcol"])
            if skip_mm:
                continue
            for b in range(w["b0"], w["b1"]):
                runs = w["mm"][b]
                r = b - w["b0"]
                nmm = sum(K for (_, _, K) in runs)
                ps = apsum.tile([128, BLK], f32)
                i = 0
                for (c, j0, K) in runs:
                    if skip_eq:
                        st = st_static
                    else:
                        st = spool.tile([128, meta["kmax"], BLK], sdt,
                                        tag="s")
                        rc = gtiles[c][1]
                        nc.vector.tensor_tensor(
                            out=st[:, :K, :],
                            in0=rel_sb[:, rc + j0:rc + j0 + K].to_broadcast(
                                [128, K, BLK]),
                            in1=iota_sb[:, r:r + 1, :].to_broadcast(
                                [128, K, BLK]),
                            op=iseq,
                        )
                    if skip_pe:
                        i += K
                        continue
                    for k in range(K):
                        nc.tensor.matmul(
                            ps[:],
                            lhsT=gtiles[c][0][:, j0 + k, :],
                            rhs=st[:, k, :],
                            start=(i == 0),
                            stop=(i == nmm - 1),
                        )
                        i += 1
                if skip_pe:
                    continue
                aggT = aggp.tile([128, BLK], f32)
                nc.any.tensor_copy(out=aggT[:], in_=ps[:])
                if skip_s2:
                    continue
                op = opsum.tile([128, O], f32)
                nc.tensor.matmul(op[:], lhsT=aggT[:], rhs=wt_sb[:],
                                 start=True, stop=True)
                nc.vector.tensor_scalar(
                    out=stage[:, b * O:(b + 1) * O],
                    in0=op[:],
                    scalar1=nrm_sb[:, b:b + 1],
                    scalar2=0.0,
                    op0=mult,
                    op1=amax,
                )
        nc.sync.dma_start(out=t_out[:], in_=stage[:])
    nc.compile()
    return nc


# ----------------------------------------------------------------- entry
TRACE = False      # set True (e.g. from test.py) to profile; result in LAST
LAST = None


def kernel(feature, norm, weight, edge_src, edge_dst):
    from concourse.bass_utils import run_bass_kernel_spmd

    meta, in_maps = _prep(feature, norm, weight, edge_src, edge_dst)
    nc = _build(meta)
    res = run_bass_kernel_spmd(nc, in_maps, list(range(NCORES)), trace=TRACE)
    globals()["LAST"] = res
    outs = []
    for k in range(NCORES):
        blocked = np.asarray(res.results[k]["out"], dtype=np.float32)
        ob = blocked.reshape(128, NBLK, O).transpose(1, 0, 2)
        outs.append(ob.reshape(NBLK * BLK, O)[:ND])
    return np.concatenate(outs, axis=0)

